# revision 32
# baseline (speedup 1.0000x reference)
"""Trainium2 SPMD kernel for nn_AutoCorrelation_loss_V (sparse_attention).

Math summary (reference reduces to this exactly):
  - scores are constant along the unmasked (causal) key range, so softmax is
    uniform over l <= index[k]: the output is cumsum(V, axis=L) with the 7
    selected rows divided by (idx+1).
  - the top-7 indices come from corr.mean(batch), where
      corr[b,t] = 0.25*(LSE_i1 + LSE_i2 + LSE_t1 + LSE_t2) - <q[b,t], k[b,t]>
    with LSE_t* = row-logsumexp (diag dropped) of the temporal Gram
    Z_b @ Z_b^T (Z_b = concat(q_b, k_b), [4096, 512]) and LSE_i* the row-LSE
    of the per-timestep 8x8 instance Gram.
  - only the top-7 RANKING feeds the output (corr values never do), so the
    temporal Gram (99.6% of the FLOPs) runs in fp8e4 with DoubleRow matmuls
    (0.5 cycles/row in the cost model, 4x cheaper than f32r) -- verified to
    preserve the exact top-7 with ~5x margin.

Sharding (8 cores): core c = (b = c//2, half = c%2), owning the 2048 Gram
rows [2048*half, +2048) of batch b:
  - own-half columns use upper-triangular symmetry, incl. 128-granular
    triangles inside the diagonal superblocks; the cross block is split
    checkerboard-style between the two cores of a batch. The true diagonal
    is subtracted exactly via a diag(|z8_r|^2/64) @ (-64*I) fp8 matmul.
  - ScalarE exps each Gram tile (bias=-SHIFT) into bf16 scr tiles -- the
    single elementwise pass over the half-Gram is the bottleneck engine.
  - row sums: DVE tensor_scalar accumulate over scr (4x bf16 mode);
    mirrored row sums: PE one-hot column-sum matmuls of scr (cross slices
    pre-folded 4->1 on DVE); combined on the host in float64.
  - cumsum of V planes (b, heads 4*half+0..3) via bf16 triangular-ones
    matmuls (chunk-local cumsum + chunk-sum carry), interleaved into PE
    slack; a PE warmup burst holds the p-state ramp off the critical path.
Host: assembles row LSEs, computes the tiny instance grams / exact dots
(0.4% of FLOPs), takes top-7, divides those rows by (idx+1) while writing
the [4, 8, 2048, 64] output.
"""

import sys

import numpy as np

sys.path.insert(0, "/opt/trn_rl_repo")

import concourse.bacc as bacc
import concourse.tile as tile
from concourse import mybir
from concourse.bass_utils import run_bass_kernel_spmd

F32 = mybir.dt.float32
BF16 = mybir.dt.bfloat16
FP8 = mybir.dt.float8e4
DR = mybir.MatmulPerfMode.DoubleRow
EXP = mybir.ActivationFunctionType.Exp
ADD = mybir.AluOpType.add

B, L, H, E = 4, 2048, 8, 64
C = H * E  # 512
T2 = 2 * L  # 4096
NCORES = 8
TOPK = 7  # int(1.0 * log(2048))
SHIFT = 100.0

PAIRS_RC = [(0, 1), (0, 2), (0, 3), (1, 2), (1, 3), (2, 3)]

LAST_RUN = None  # BassKernelResults of the most recent launch (for test.py)
_CACHED = {}


def _build_nc():
    nc = bacc.Bacc("TRN2", target_bir_lowering=False, debug=False,
                   num_devices=NCORES)

    zdr_d = nc.dram_tensor("zdr", [2, 128, 2, T2], FP8, kind="ExternalInput").ap()
    dcan_d = nc.dram_tensor("dcan", [128, 2, L], FP8, kind="ExternalInput").ap()
    negi_d = nc.dram_tensor("negi", [128, 2, 128], FP8, kind="ExternalInput").ap()
    vp_d = nc.dram_tensor("vp", [128, 16, 4, E], BF16, kind="ExternalInput").ap()
    triu_d = nc.dram_tensor("triu", [128, 128], BF16, kind="ExternalInput").ap()
    ltw_d = nc.dram_tensor("ltw", [16, L], BF16, kind="ExternalInput").ap()
    ohwb_d = nc.dram_tensor("ohwb", [128, 63], BF16, kind="ExternalInput").ap()
    warm_d = nc.dram_tensor("warm", [128, 512], BF16, kind="ExternalInput").ap()

    esums_d = nc.dram_tensor("esums", [128, 16, 2], F32, kind="ExternalOutput").ap()
    csums_d = nc.dram_tensor("csums", [32, 512], F32, kind="ExternalOutput").ap()
    csumsb_d = nc.dram_tensor("csumsb", [32, 512], F32, kind="ExternalOutput").ap()
    planes_d = nc.dram_tensor("planes", [128, 16, 256], F32, kind="ExternalOutput").ap()

    with tile.TileContext(nc) as tc:
        with tc.tile_pool(name="zin", bufs=1) as zp, \
             tc.tile_pool(name="const", bufs=1) as cp, \
             tc.tile_pool(name="small", bufs=1) as smp, \
             tc.tile_pool(name="scra", bufs=5) as sca, \
             tc.tile_pool(name="scrb", bufs=5) as scb, \
             tc.tile_pool(name="fold", bufs=1) as fop, \
             tc.tile_pool(name="junk", bufs=2) as jkp, \
             tc.tile_pool(name="pout", bufs=2) as pop, \
             tc.tile_pool(name="gram", bufs=2, space="PSUM") as gp, \
             tc.tile_pool(name="csp", bufs=1, space="PSUM") as csp, \
             tc.tile_pool(name="cum", bufs=1, space="PSUM") as cup:

            # ---- inputs, all on the SP HWDGE queue so the global DMA
            # device processes them in exactly this order ----
            warm_sb = cp.tile([128, 512], BF16, tag="warm")
            nc.sync.dma_start(warm_sb[:], warm_d)
            bias_sb = cp.tile([128, 1], F32, tag="bias")
            nc.gpsimd.memset(bias_sb[:], -SHIFT)

            zdr_sb = []
            for j in range(2):
                t = zp.tile([128, 2, T2], FP8, tag=f"zdr{j}", name=f"zdr{j}")
                zdr_sb.append(t)
            for j in range(2):  # own columns first
                nc.sync.dma_start(zdr_sb[j][:, :, 0:2048],
                                  zdr_d[j][:, :, 0:2048])
            for j in range(2):  # cross chunks of even superblocks
                nc.sync.dma_start(zdr_sb[j][:, :, 2048:3072],
                                  zdr_d[j][:, :, 2048:3072])
            dcan_sb = zp.tile([128, 2, L], FP8, tag="dcan")
            nc.sync.dma_start(dcan_sb[:], dcan_d)
            negi_sb = zp.tile([128, 2, 128], FP8, tag="negi")
            nc.sync.dma_start(negi_sb[:], negi_d)
            ohwb_sb = cp.tile([128, 63], BF16, tag="ohwb")
            nc.sync.dma_start(ohwb_sb[:], ohwb_d)
            for j in range(2):  # cross chunks of odd superblocks
                nc.sync.dma_start(zdr_sb[j][:, :, 3072:4096],
                                  zdr_d[j][:, :, 3072:4096])
            vp_sb = zp.tile([128, 16, 4, E], BF16, tag="vp")
            nc.sync.dma_start(vp_sb[:], vp_d)
            triu_sb = cp.tile([128, 128], BF16, tag="triu")
            nc.sync.dma_start(triu_sb[:], triu_d)
            ltw_sb = cp.tile([16, L], BF16, tag="ltw")
            nc.sync.dma_start(ltw_sb[:], ltw_d)

            esums_sb = smp.tile([128, 16, 2], F32, tag="esums")
            nc.gpsimd.memset(esums_sb[:], 0.0)
            cs_ps = csp.tile([32, 512], F32, tag="csps")
            cs_b = None  # late-created [32, 512] in the cum bank for g=3 cross
            # PE warmup: ramp the p-state while zdr streams in (output is
            # never read; the slot is recycled by the first gram tile)
            ps_warm = gp.tile([128, 1536], F32, tag="gram", name="ps_warm")
            for _ in range(6):
                nc.tensor.matmul(ps_warm[:, 0:512], warm_sb[:, 0:128],
                                 warm_sb[:], start=True, stop=True,
                                 skip_group_check=True)
            cs_total = 42
            cs_state = {"n": 0}

            def cs_mm(row, rhs_ap, col0=0):
                # accumulate column sums of rhs into cs_ps[row, col0:]
                w = rhs_ap.shape[-1]
                nc.tensor.matmul(cs_ps[:, col0:col0 + w],
                                 ohwb_sb[:, 31 - row:63 - row], rhs_ap,
                                 start=cs_state["n"] == 0,
                                 stop=cs_state["n"] == cs_total - 1,
                                 skip_group_check=True)
                cs_state["n"] += 1

            csb_state = {"n": 0}

            def cs_mm_b(row, rhs_ap):
                # g=3 cross colsums into the late cs_b accumulator
                nc.tensor.matmul(cs_b[:, 0:512],
                                 ohwb_sb[:, 31 - row:63 - row], rhs_ap,
                                 start=csb_state["n"] == 0,
                                 stop=csb_state["n"] == 7,
                                 skip_group_check=True)
                csb_state["n"] += 1

            def gram_mms(ps, col0, lcol, rcol, w, cancel=False):
                # accumulate Z8[:, lcol:+128]^T @ Z8[:, rcol:+w] into
                # ps[:, col0:col0+w]; optionally subtract the true diagonal
                # (diag(|z8_r|^2/64) @ (-64 I), exact to ~|20| which the
                # exp(-SHIFT) bias flushes to zero)
                nc.tensor.matmul(ps[:, col0:col0 + w],
                                 zdr_sb[0][:, :, lcol:lcol + 128],
                                 zdr_sb[0][:, :, rcol:rcol + w],
                                 start=True, stop=False, perf_mode=DR)
                if cancel:
                    nc.tensor.matmul(ps[:, col0:col0 + 128],
                                     dcan_sb[:, :, lcol:lcol + 128],
                                     negi_sb[:],
                                     start=False, stop=False, perf_mode=DR,
                                     skip_group_check=True)
                nc.tensor.matmul(ps[:, col0:col0 + w],
                                 zdr_sb[1][:, :, lcol:lcol + 128],
                                 zdr_sb[1][:, :, rcol:rcol + w],
                                 start=False, stop=True, perf_mode=DR)

            def chunk_sums():
                # V chunk sums for the cumsum carries
                ps_sums = cup.tile([16, 256], F32, tag="pc", name="ps_sums")
                for n in range(16):
                    nc.tensor.matmul(ps_sums[:], ohwb_sb[:, 31 - n:47 - n],
                                     vp_sb[:, n], start=(n == 0),
                                     stop=(n == 15))
                sums_sb = smp.tile([16, 256], BF16, tag="sums_sb")
                nc.vector.tensor_copy(sums_sb[:], ps_sums[:])
                return sums_sb

            sums_sb = None

            def cumsum_round(nr):
                # chunks (2nr, 2nr+1) -> planes
                pc = cup.tile([128, 2, 256], F32, tag="pc", name="pc")
                for hh in range(2):
                    n = 2 * nr + hh
                    nc.tensor.matmul(pc[:, hh, :],
                                     ltw_sb[:, 128 * n:128 * n + 128],
                                     sums_sb[:], start=True, stop=False)
                    nc.tensor.matmul(pc[:, hh, :], triu_sb[:],
                                     vp_sb[:, n], start=False, stop=True)
                po = pop.tile([128, 2, 256], F32, tag="pout")
                nc.vector.tensor_copy(po[:], pc[:])
                nc.sync.dma_start(planes_d[:, 2 * nr:2 * nr + 2], po[:])

            # ---- the Gram m-loop ----
            # Tile column layouts (all matmul regions 512-bank-aligned; the
            # diag part sits at its natural [128i, 512) offset in bank 0):
            #   g=0: A = [chk1 | chk2 | chk3]          B = [diag | crossx2]
            #   g=1: A = [diag | chk2 | chk3]          B = [crossx2]
            #   g=2: A = [diag | chk3]                 B = [crossx2]
            #   g=3: A = [diag | crossx2]              (no B)
            sb_scr_all = {g: [] for g in range(4)}
            fold_state = {}

            def fold_tiles(g):
                # per-superblock bf16 cross-fold accumulator on DVE
                fc = fop.tile([128, 1024], BF16, tag=f"fc{g}", name=f"fc{g}")
                fold_state[g] = fc

            def fold_tile(g, i):
                # accumulate m-tile (g, i)'s cross slice (g=3 goes direct)
                if g == 3:
                    return
                scr_a, scr_b = sb_scr_all[g][i]
                fc = fold_state[g]
                cscr, co = (scr_b, 512) if g == 0 else (scr_b, 0)
                csrc = cscr[:, co:co + 1024]
                if i == 0:
                    nc.vector.tensor_copy(fc[:], csrc)
                else:
                    nc.vector.tensor_add(fc[:], fc[:], csrc)

            def reduce_tile(g, i):
                # diag partials and own-pair colsums direct from scr (PE);
                # the folded cross colsums at i == 3
                scr_a, scr_b = sb_scr_all[g][i]
                for n in range(g + 1, 4):
                    o = 512 * (n - 1) if g == 0 else 512 * (n - g)
                    cs_mm(PAIRS_RC.index((g, n)), scr_a[:, o:o + 512])
                if i < 3:
                    dscr = scr_b if g == 0 else scr_a
                    cs_mm(14 + 3 * g + i, dscr[:, 128 * i:512], col0=128 * i)
                if g == 3:
                    for hb in range(2):
                        cs_mm_b(6 + 2 * g + hb,
                                scr_a[:, 512 + 512 * hb:1024 + 512 * hb])
                elif i == 3:
                    fc = fold_state[g]
                    for hb in range(2):
                        cs_mm(6 + 2 * g + hb, fc[:, 512 * hb:512 * hb + 512])

            for g in range(4):
                cpos = [2 * (g % 2), 2 * (g % 2) + 1]  # stored cross slots
                sb_scr = sb_scr_all[g]
                for i in range(4):
                    m = 4 * g + i
                    r0 = 128 * m
                    dia_c = 512 * g + 128 * i  # first diag-part column
                    dia_w = 512 - 128 * i
                    ps_a = gp.tile([128, 1536], F32, tag="gram", name="ps_a")
                    ps_b = None
                    if g == 0:
                        for n in range(1, 4):
                            gram_mms(ps_a, 512 * (n - 1), r0, 512 * n, 512)
                        s_a, e_a = 0, 1536
                        ps_b = gp.tile([128, 1536], F32, tag="gram", name="ps_b")
                        for hb in range(2):
                            gram_mms(ps_b, 512 + 512 * hb, r0,
                                     2048 + 512 * cpos[hb], 512)
                        gram_mms(ps_b, 128 * i, r0, dia_c, dia_w, cancel=True)
                        s_b, e_b = 128 * i, 1536
                    elif g < 3:
                        gram_mms(ps_a, 128 * i, r0, dia_c, dia_w, cancel=True)
                        for n in range(g + 1, 4):
                            gram_mms(ps_a, 512 * (n - g), r0, 512 * n, 512)
                        s_a, e_a = 128 * i, 512 * (4 - g)
                        ps_b = gp.tile([128, 1536], F32, tag="gram", name="ps_b")
                        for hb in range(2):
                            gram_mms(ps_b, 512 * hb, r0,
                                     2048 + 512 * cpos[hb], 512)
                        s_b, e_b = 0, 1024
                    else:
                        gram_mms(ps_a, 128 * i, r0, dia_c, dia_w, cancel=True)
                        for hb in range(2):
                            gram_mms(ps_a, 512 + 512 * hb, r0,
                                     2048 + 512 * cpos[hb], 512)
                        s_a, e_a = 128 * i, 1536

                    scr_a = sca.tile([128, 1536], BF16, tag="scr_a")
                    nc.scalar.activation(scr_a[:, s_a:e_a], ps_a[:, s_a:e_a],
                                         EXP, bias=bias_sb[:])
                    jk = jkp.tile([128, 1536], BF16, tag="jk")
                    nc.vector.tensor_scalar(jk[:, s_a:e_a],
                                            scr_a[:, s_a:e_a], 0.0, None,
                                            op0=ADD, op1=ADD,
                                            accum_out=esums_sb[:, m, 0:1])
                    scr_b = None
                    if ps_b is not None:
                        scr_b = scb.tile([128, 1536], BF16, tag="scr_b")
                        nc.scalar.activation(scr_b[:, s_b:e_b], ps_b[:, s_b:e_b],
                                             EXP, bias=bias_sb[:])
                        jk2 = jkp.tile([128, 1536], BF16, tag="jk")
                        nc.vector.tensor_scalar(jk2[:, s_b:e_b],
                                                scr_b[:, s_b:e_b], 0.0, None,
                                                op0=ADD, op1=ADD,
                                                accum_out=esums_sb[:, m, 1:2])
                    sb_scr.append((scr_a, scr_b))
                    if i == 0 and g < 3:
                        fold_tiles(g)
                    fold_tile(g, i)

                    # deferred work, spread so the PE never stalls in a clump
                    if m > 0:
                        reduce_tile((m - 1) // 4, (m - 1) % 4)
                    if m == 3:
                        sums_sb = chunk_sums()
                    if 5 <= m <= 12:
                        cumsum_round(m - 5)
                    if m == 12:
                        cs_b = cup.tile([32, 512], F32, tag="pc",
                                        name="cs_b")

            # ---- tail: flush cs_a (complete once m14's pieces are in),
            # then the last m-tile's colsums into cs_b ----
            csums_sb = smp.tile([32, 512], F32, tag="csums_sb")
            nc.vector.tensor_copy(csums_sb[:], cs_ps[:])
            nc.sync.dma_start(csums_d, csums_sb[:])
            reduce_tile(3, 3)
            csb_sb = smp.tile([32, 512], F32, tag="csb_sb")
            nc.vector.tensor_copy(csb_sb[:], cs_b[:, 0:512])
            nc.sync.dma_start(csumsb_d, csb_sb[:])
            nc.sync.dma_start(esums_d, esums_sb[:])

    nc.compile()
    return nc


def _consts():
    import ml_dtypes
    kk = np.arange(128)
    triu = (kk[:, None] <= kk[None, :]).astype(ml_dtypes.bfloat16)
    cc = np.arange(16)
    nn = np.arange(L) // 128
    ltw = (cc[:, None] < nn[None, :]).astype(ml_dtypes.bfloat16)
    ohwb = np.zeros((128, 63), np.float32)
    ohwb[:, 31] = 1.0
    return triu, ltw, ohwb.astype(ml_dtypes.bfloat16)


def prepare_in_maps(queries, keys, values):
    import ml_dtypes
    BF = ml_dtypes.bfloat16
    F8 = ml_dtypes.float8_e4m3fn
    q = np.ascontiguousarray(queries, dtype=np.float32).reshape(B, L, C)
    k = np.ascontiguousarray(keys, dtype=np.float32).reshape(B, L, C)
    v = np.ascontiguousarray(values, dtype=np.float32)            # [B,L,H,E]

    triu, ltw, ohwb = _consts()

    in_maps = []
    for c in range(NCORES):
        b, half = c // 2, c % 2
        Zb = np.concatenate([q[b], k[b]], axis=0)                 # [4096, 512]
        own = Zb[L * half:L * half + L]
        oth = Zb[L * (1 - half):L * (1 - half) + L]
        # rotate other-half 512-blocks by `half` so the checkerboard covers
        # complementary cross sub-blocks on the two cores of a batch
        oth = np.concatenate(
            [oth[512 * ((i + half) % 4):512 * ((i + half) % 4) + 512]
             for i in (0, 2, 1, 3)], axis=0)
        z8 = np.concatenate([own, oth], axis=0).astype(F8)        # [4096, 512]
        # zdr[j][p, kt, t] = z8[t, 256j + 128kt + p]
        zdr = np.zeros((2, 128, 2, T2), dtype=F8)
        for j in range(2):
            for kt in range(2):
                c0 = 256 * j + 128 * kt
                zdr[j, :, kt, :] = z8[:, c0:c0 + 128].T
        # diagonal cancel: dcan[c, 0, r] = delta_{c, r%128} |z8_r|^2 / 64
        d = (z8[:L].astype(np.float32) ** 2).sum(axis=1) / 64.0   # [2048]
        dcan = np.zeros((128, 2, L), dtype=F8)
        rr_ = np.arange(L)
        dcan[rr_ % 128, 0, rr_] = d.astype(F8)
        negi = np.zeros((128, 2, 128), dtype=F8)
        negi[np.arange(128), 0, np.arange(128)] = -64.0

        vp = np.ascontiguousarray(
            v[b].reshape(16, 128, H, E)[:, :, 4 * half:4 * half + 4, :]
            .transpose(1, 0, 2, 3))                               # [128,16,4,64]
        in_maps.append({
            "zdr": zdr, "dcan": dcan, "negi": negi,
            "vp": vp.astype(BF), "warm": np.zeros((128, 512), dtype=BF),
            "triu": triu, "ltw": ltw, "ohwb": ohwb,
        })
    return in_maps


def get_nc():
    if "nc" not in _CACHED:
        _CACHED["nc"] = _build_nc()
    return _CACHED["nc"]


def kernel(queries, keys, values, attn_mask):
    global LAST_RUN
    nc = get_nc()
    in_maps = prepare_in_maps(queries, keys, values)

    res = run_bass_kernel_spmd(nc, in_maps, list(range(NCORES)))
    LAST_RUN = res
    return _host_combine(res.results, queries, keys)


def _host_combine(results, queries, keys):
    # ---- host combine (tiny, float64) ----
    srows = np.zeros((B, 2, L))
    for c in range(NCORES):
        b, half = c // 2, c % 2
        r = results[c]
        es = r["esums"].astype(np.float64)                        # [128, 16, 2]
        srow = (es[:, :, 0] + es[:, :, 1]).T.reshape(L).copy()    # r = 128m+p
        cs = r["csums"].astype(np.float64)                        # [32, 512]
        cs[12:14] = r["csumsb"].astype(np.float64)[12:14]
        for p_idx, (rr, ccol) in enumerate(PAIRS_RC):
            srow[512 * ccol:512 * ccol + 512] += cs[p_idx]
        for g in range(4):
            for ip in range(3):
                x0 = 128 * (ip + 1)
                srow[512 * g + x0:512 * g + 512] += cs[14 + 3 * g + ip, x0:]
        srows[b, half] += srow
        # cross checkerboard colsums belong to the sibling core's rows
        for rp in range(4):
            for hb in range(2):
                cact = ((rp % 2 + 2 * hb) + half) % 4
                srows[b, 1 - half, 512 * cact:512 * cact + 512] += \
                    cs[6 + 2 * rp + hb]
    lse_t_sum = (np.log(srows) + SHIFT).sum(axis=(0, 1))          # [L]

    # instance part: per-timestep 8x8 grams (0.2% of the FLOPs) + exact dots
    q = queries.astype(np.float32).reshape(B, L, C)
    k = keys.astype(np.float32).reshape(B, L, C)
    Zi = np.concatenate([q, k], axis=0).transpose(1, 0, 2)        # [L, 2B, C]
    e = np.einsum('tic,tjc->tij', Zi, Zi).astype(np.float64)      # [L, 8, 8]
    dots = np.stack([e[:, bb, B + bb] for bb in range(B)])        # [B, L]
    ii = np.arange(2 * B)
    e[:, ii, ii] = -np.inf
    m = e.max(axis=2, keepdims=True)
    li_sum = (np.log(np.exp(e - m).sum(axis=2)) + m[..., 0]).sum(axis=1)

    corr_mean = (li_sum + lse_t_sum) / 16.0 - dots.mean(axis=0)
    index = np.argsort(-corr_mean, kind="stable")[:TOPK]

    out = np.empty((B, H, L, E), np.float32)
    for c in range(NCORES):
        b, half = c // 2, c % 2
        pl = results[c]["planes"].reshape(128, 16, 4, E)
        out[b, 4 * half:4 * half + 4] = (
            pl.transpose(2, 1, 0, 3).reshape(4, L, E))
    out[:, :, index, :] /= (index + 1).astype(np.float32)[None, None, :, None]
    return out


# revision 38
# speedup vs baseline: 1.0363x; 1.0363x over previous
"""Trainium2 SPMD kernel for nn_AutoCorrelation_loss_V (sparse_attention).

Math summary (reference reduces to this exactly):
  - scores are constant along the unmasked (causal) key range, so softmax is
    uniform over l <= index[k]: the output is cumsum(V, axis=L) with the 7
    selected rows divided by (idx+1).
  - the top-7 indices come from corr.mean(batch), where
      corr[b,t] = 0.25*(LSE_i1 + LSE_i2 + LSE_t1 + LSE_t2) - <q[b,t], k[b,t]>
    with LSE_t* = row-logsumexp (diag dropped) of the temporal Gram
    Z_b @ Z_b^T (Z_b = concat(q_b, k_b), [4096, 512]) and LSE_i* the row-LSE
    of the per-timestep 8x8 instance Gram.
  - only the top-7 RANKING feeds the output (corr values never do), so the
    temporal Gram (99.6% of the FLOPs) runs in fp8e4 with DoubleRow matmuls
    (0.5 cycles/row in the cost model, 4x cheaper than f32r) -- verified to
    preserve the exact top-7 with ~5x margin.

Sharding (8 cores): core c = (b = c//2, half = c%2), owning the 2048 Gram
rows [2048*half, +2048) of batch b:
  - own-half columns use upper-triangular symmetry, incl. 128-granular
    triangles inside the diagonal superblocks; the cross block is split
    checkerboard-style between the two cores of a batch. The true diagonal
    is subtracted exactly via a diag(|z8_r|^2/64) @ (-64*I) fp8 matmul.
  - ScalarE exps each Gram tile (bias=-SHIFT) into bf16 scr tiles -- the
    single elementwise pass over the half-Gram is the bottleneck engine.
  - row sums: DVE tensor_scalar accumulate over scr (4x bf16 mode);
    mirrored row sums: PE one-hot column-sum matmuls of scr (cross slices
    pre-folded 4->1 on DVE); combined on the host in float64.
  - cumsum of V planes (b, heads 4*half+0..3) via bf16 triangular-ones
    matmuls (chunk-local cumsum + chunk-sum carry), interleaved into PE
    slack; a PE warmup burst holds the p-state ramp off the critical path.
Host: assembles row LSEs, computes the tiny instance grams / exact dots
(0.4% of FLOPs), takes top-7, divides those rows by (idx+1) while writing
the [4, 8, 2048, 64] output.
"""

import sys

import numpy as np

sys.path.insert(0, "/opt/trn_rl_repo")

import concourse.bacc as bacc
import concourse.tile as tile
from concourse import mybir
from concourse.bass_utils import run_bass_kernel_spmd

F32 = mybir.dt.float32
BF16 = mybir.dt.bfloat16
FP8 = mybir.dt.float8e4
DR = mybir.MatmulPerfMode.DoubleRow
EXP = mybir.ActivationFunctionType.Exp
ADD = mybir.AluOpType.add

B, L, H, E = 4, 2048, 8, 64
C = H * E  # 512
T2 = 2 * L  # 4096
NCORES = 8
TOPK = 7  # int(1.0 * log(2048))
SHIFT = 100.0

PAIRS_RC = [(0, 1), (0, 2), (0, 3), (1, 2), (1, 3), (2, 3)]

LAST_RUN = None  # BassKernelResults of the most recent launch (for test.py)
_CACHED = {}


def _build_nc():
    nc = bacc.Bacc("TRN2", target_bir_lowering=False, debug=False,
                   num_devices=NCORES)

    zdr_d = nc.dram_tensor("zdr", [2, 128, 2, T2], FP8, kind="ExternalInput").ap()
    dcan_d = nc.dram_tensor("dcan", [128, 2, L], FP8, kind="ExternalInput").ap()
    negi_d = nc.dram_tensor("negi", [128, 2, 128], FP8, kind="ExternalInput").ap()
    vp_d = nc.dram_tensor("vp", [128, 16, 4, E], BF16, kind="ExternalInput").ap()
    triu_d = nc.dram_tensor("triu", [128, 128], BF16, kind="ExternalInput").ap()
    ltw_d = nc.dram_tensor("ltw", [16, L], BF16, kind="ExternalInput").ap()
    ohwb_d = nc.dram_tensor("ohwb", [128, 63], BF16, kind="ExternalInput").ap()

    esums_d = nc.dram_tensor("esums", [128, 16, 2], F32, kind="ExternalOutput").ap()
    csums_d = nc.dram_tensor("csums", [32, 512], F32, kind="ExternalOutput").ap()
    csumsb_d = nc.dram_tensor("csumsb", [32, 512], F32, kind="ExternalOutput").ap()
    planes_d = nc.dram_tensor("planes", [128, 16, 256], F32, kind="ExternalOutput").ap()

    with tile.TileContext(nc) as tc:
        with tc.tile_pool(name="zin", bufs=1) as zp, \
             tc.tile_pool(name="const", bufs=1) as cp, \
             tc.tile_pool(name="small", bufs=1) as smp, \
             tc.tile_pool(name="scra", bufs=5) as sca, \
             tc.tile_pool(name="scrb", bufs=5) as scb, \
             tc.tile_pool(name="fold", bufs=1) as fop, \
             tc.tile_pool(name="junk", bufs=2) as jkp, \
             tc.tile_pool(name="pout", bufs=2) as pop, \
             tc.tile_pool(name="gram", bufs=2, space="PSUM") as gp, \
             tc.tile_pool(name="csp", bufs=1, space="PSUM") as csp, \
             tc.tile_pool(name="cum", bufs=1, space="PSUM") as cup:

            # ---- inputs, all on the SP HWDGE queue so the global DMA
            # device processes them in exactly this order ----
            warm_sb = cp.tile([128, 512], BF16, tag="warm")
            nc.gpsimd.memset(warm_sb[:], 0.0)
            bias_sb = cp.tile([128, 1], F32, tag="bias")
            nc.gpsimd.memset(bias_sb[:], -SHIFT)
            # dummy exp so Bacc's activation-table load runs during the
            # DMA-wait window instead of delaying the first real activation
            dummy_sb = cp.tile([128, 1], BF16, tag="dummy")
            nc.scalar.activation(dummy_sb[:], bias_sb[:], EXP, bias=bias_sb[:])

            zdr_sb = []
            for j in range(2):
                t = zp.tile([128, 2, T2], FP8, tag=f"zdr{j}", name=f"zdr{j}")
                zdr_sb.append(t)
            for j in range(2):  # own columns first
                nc.sync.dma_start(zdr_sb[j][:, :, 0:2048],
                                  zdr_d[j][:, :, 0:2048])
            for j in range(2):  # cross chunks of even superblocks
                nc.sync.dma_start(zdr_sb[j][:, :, 2048:3072],
                                  zdr_d[j][:, :, 2048:3072])
            dcan_sb = zp.tile([128, 2, L], FP8, tag="dcan")
            nc.sync.dma_start(dcan_sb[:], dcan_d)
            negi_sb = zp.tile([128, 2, 128], FP8, tag="negi")
            nc.sync.dma_start(negi_sb[:], negi_d)
            ohwb_sb = cp.tile([128, 63], BF16, tag="ohwb")
            nc.sync.dma_start(ohwb_sb[:], ohwb_d)
            for j in range(2):  # cross chunks of odd superblocks
                nc.sync.dma_start(zdr_sb[j][:, :, 3072:4096],
                                  zdr_d[j][:, :, 3072:4096])
            vp_sb = zp.tile([128, 16, 4, E], BF16, tag="vp")
            nc.sync.dma_start(vp_sb[:], vp_d)
            triu_sb = cp.tile([128, 128], BF16, tag="triu")
            nc.sync.dma_start(triu_sb[:], triu_d)
            ltw_sb = cp.tile([16, L], BF16, tag="ltw")
            nc.sync.dma_start(ltw_sb[:], ltw_d)

            esums_sb = smp.tile([128, 16, 2], F32, tag="esums")
            nc.gpsimd.memset(esums_sb[:], 0.0)
            cs_ps = csp.tile([32, 512], F32, tag="csps")
            cs_b = None  # late-created [32, 512] in the cum bank for g=3 cross
            # PE warmup: ramp the p-state while zdr streams in (output is
            # never read; the slot is recycled by the first gram tile)
            ps_warm = gp.tile([128, 1536], F32, tag="gram", name="ps_warm")
            for _ in range(6):
                nc.tensor.matmul(ps_warm[:, 0:512], warm_sb[:, 0:128],
                                 warm_sb[:], start=True, stop=True,
                                 skip_group_check=True)
            cs_total = 42
            cs_state = {"n": 0}

            def cs_mm(row, rhs_ap, col0=0):
                # accumulate column sums of rhs into cs_ps[row, col0:]
                w = rhs_ap.shape[-1]
                nc.tensor.matmul(cs_ps[:, col0:col0 + w],
                                 ohwb_sb[:, 31 - row:63 - row], rhs_ap,
                                 start=cs_state["n"] == 0,
                                 stop=cs_state["n"] == cs_total - 1,
                                 skip_group_check=True)
                cs_state["n"] += 1

            csb_state = {"n": 0}

            def cs_mm_b(row, rhs_ap):
                # g=3 cross colsums into the late cs_b accumulator
                nc.tensor.matmul(cs_b[:, 0:512],
                                 ohwb_sb[:, 31 - row:63 - row], rhs_ap,
                                 start=csb_state["n"] == 0,
                                 stop=csb_state["n"] == 7,
                                 skip_group_check=True)
                csb_state["n"] += 1

            def gram_mms(ps, col0, lcol, rcol, w, cancel=False):
                # accumulate Z8[:, lcol:+128]^T @ Z8[:, rcol:+w] into
                # ps[:, col0:col0+w]; optionally subtract the true diagonal
                # (diag(|z8_r|^2/64) @ (-64 I), exact to ~|20| which the
                # exp(-SHIFT) bias flushes to zero)
                nc.tensor.matmul(ps[:, col0:col0 + w],
                                 zdr_sb[0][:, :, lcol:lcol + 128],
                                 zdr_sb[0][:, :, rcol:rcol + w],
                                 start=True, stop=False, perf_mode=DR)
                if cancel:
                    nc.tensor.matmul(ps[:, col0:col0 + 128],
                                     dcan_sb[:, :, lcol:lcol + 128],
                                     negi_sb[:],
                                     start=False, stop=False, perf_mode=DR,
                                     skip_group_check=True)
                nc.tensor.matmul(ps[:, col0:col0 + w],
                                 zdr_sb[1][:, :, lcol:lcol + 128],
                                 zdr_sb[1][:, :, rcol:rcol + w],
                                 start=False, stop=True, perf_mode=DR)

            def chunk_sums():
                # V chunk sums for the cumsum carries
                ps_sums = cup.tile([16, 256], F32, tag="pc", name="ps_sums")
                for n in range(16):
                    nc.tensor.matmul(ps_sums[:], ohwb_sb[:, 31 - n:47 - n],
                                     vp_sb[:, n], start=(n == 0),
                                     stop=(n == 15))
                sums_sb = smp.tile([16, 256], BF16, tag="sums_sb")
                nc.vector.tensor_copy(sums_sb[:], ps_sums[:])
                return sums_sb

            sums_sb = None

            def cumsum_round(nr):
                # chunks (2nr, 2nr+1) -> planes
                pc = cup.tile([128, 2, 256], F32, tag="pc", name="pc")
                for hh in range(2):
                    n = 2 * nr + hh
                    nc.tensor.matmul(pc[:, hh, :],
                                     ltw_sb[:, 128 * n:128 * n + 128],
                                     sums_sb[:], start=True, stop=False)
                    nc.tensor.matmul(pc[:, hh, :], triu_sb[:],
                                     vp_sb[:, n], start=False, stop=True)
                po = pop.tile([128, 2, 256], F32, tag="pout")
                nc.vector.tensor_copy(po[:], pc[:])
                nc.sync.dma_start(planes_d[:, 2 * nr:2 * nr + 2], po[:])

            # ---- the Gram m-loop ----
            # Tile column layouts (all matmul regions 512-bank-aligned; the
            # diag part sits at its natural [128i, 512) offset in bank 0):
            #   g=0: A = [chk1 | chk2 | chk3]          B = [diag | crossx2]
            #   g=1: A = [diag | chk2 | chk3]          B = [crossx2]
            #   g=2: A = [diag | chk3]                 B = [crossx2]
            #   g=3: A = [diag | crossx2]              (no B)
            sb_scr_all = {g: [] for g in range(4)}
            fold_state = {}

            def fold_tiles(g):
                # per-superblock bf16 cross-fold accumulator on DVE
                fc = fop.tile([128, 1024], BF16, tag=f"fc{g}", name=f"fc{g}")
                fold_state[g] = fc

            def fold_tile(g, i):
                # accumulate m-tile (g, i)'s cross slice (g=3 goes direct)
                if g == 3:
                    return
                scr_a, scr_b = sb_scr_all[g][i]
                fc = fold_state[g]
                cscr, co = (scr_b, 512) if g == 0 else (scr_b, 0)
                csrc = cscr[:, co:co + 1024]
                if i == 0:
                    nc.vector.tensor_copy(fc[:], csrc)
                else:
                    nc.vector.tensor_add(fc[:], fc[:], csrc)

            def reduce_tile(g, i):
                # diag partials and own-pair colsums direct from scr (PE);
                # the folded cross colsums at i == 3
                scr_a, scr_b = sb_scr_all[g][i]
                for n in range(g + 1, 4):
                    o = 512 * (n - 1) if g == 0 else 512 * (n - g)
                    cs_mm(PAIRS_RC.index((g, n)), scr_a[:, o:o + 512])
                if i < 3:
                    dscr = scr_b if g == 0 else scr_a
                    cs_mm(14 + 3 * g + i, dscr[:, 128 * i:512], col0=128 * i)
                if g == 3:
                    for hb in range(2):
                        cs_mm_b(6 + 2 * g + hb,
                                scr_a[:, 512 + 512 * hb:1024 + 512 * hb])
                elif i == 3:
                    fc = fold_state[g]
                    for hb in range(2):
                        cs_mm(6 + 2 * g + hb, fc[:, 512 * hb:512 * hb + 512])

            def emit_act(scr, ps, s, e, m, slot):
                nc.scalar.activation(scr[:, s:e], ps[:, s:e], EXP,
                                     bias=bias_sb[:])
                jk = jkp.tile([128, 1536], BF16, tag="jk", name="jk")
                nc.vector.tensor_scalar(jk[:, s:e], scr[:, s:e], 0.0, None,
                                        op0=ADD, op1=ADD,
                                        accum_out=esums_sb[:, m, slot:slot + 1])

            def emit_b0(i):
                # g=0 B tile: [diag | cross x2] (cross mms first; the
                # dcan-gated diag cancel comes last)
                r0 = 128 * i
                ps_b = gp.tile([128, 1536], F32, tag="gram", name="ps_b")
                for hb in range(2):
                    gram_mms(ps_b, 512 + 512 * hb, r0, 2048 + 512 * hb, 512)
                gram_mms(ps_b, 128 * i, r0, 128 * i, 512 - 128 * i,
                         cancel=True)
                scr_b = scb.tile([128, 1536], BF16, tag="scr_b")
                emit_act(scr_b, ps_b, 128 * i, 1536, i, 1)
                sb_scr_all[0][i] = (sb_scr_all[0][i][0], scr_b)
                if i == 0:
                    fold_tiles(0)
                fold_tile(0, i)

            # g=0: own-only A tiles run ahead of the (cross+dcan)-gated B
            # tiles -- A0 A1 B0 A2 B1 A3 B2 [B3 at g=1 start] -- so the ACT
            # stream starts as soon as the own zdr columns land.
            for i in range(4):
                r0 = 128 * i
                ps_a = gp.tile([128, 1536], F32, tag="gram", name="ps_a")
                for n in range(1, 4):
                    gram_mms(ps_a, 512 * (n - 1), r0, 512 * n, 512)
                scr_a = sca.tile([128, 1536], BF16, tag="scr_a")
                emit_act(scr_a, ps_a, 0, 1536, i, 0)
                sb_scr_all[0].append((scr_a, None))
                if i >= 1:
                    emit_b0(i - 1)
                if i >= 2:
                    reduce_tile(0, i - 2)
                if i == 3:
                    sums_sb = chunk_sums()
            emit_b0(3)
            reduce_tile(0, 2)

            for g in range(1, 4):
                cpos = [2 * (g % 2), 2 * (g % 2) + 1]  # stored cross slots
                sb_scr = sb_scr_all[g]
                for i in range(4):
                    m = 4 * g + i
                    r0 = 128 * m
                    dia_c = 512 * g + 128 * i  # first diag-part column
                    dia_w = 512 - 128 * i
                    ps_a = gp.tile([128, 1536], F32, tag="gram", name="ps_a")
                    ps_b = None
                    if g < 3:
                        gram_mms(ps_a, 128 * i, r0, dia_c, dia_w, cancel=True)
                        for n in range(g + 1, 4):
                            gram_mms(ps_a, 512 * (n - g), r0, 512 * n, 512)
                        s_a, e_a = 128 * i, 512 * (4 - g)
                        ps_b = gp.tile([128, 1536], F32, tag="gram", name="ps_b")
                        for hb in range(2):
                            gram_mms(ps_b, 512 * hb, r0,
                                     2048 + 512 * cpos[hb], 512)
                        s_b, e_b = 0, 1024
                    else:
                        gram_mms(ps_a, 128 * i, r0, dia_c, dia_w, cancel=True)
                        for hb in range(2):
                            gram_mms(ps_a, 512 + 512 * hb, r0,
                                     2048 + 512 * cpos[hb], 512)
                        s_a, e_a = 128 * i, 1536

                    scr_a = sca.tile([128, 1536], BF16, tag="scr_a")
                    emit_act(scr_a, ps_a, s_a, e_a, m, 0)
                    scr_b = None
                    if ps_b is not None:
                        scr_b = scb.tile([128, 1536], BF16, tag="scr_b")
                        emit_act(scr_b, ps_b, s_b, e_b, m, 1)
                    sb_scr.append((scr_a, scr_b))
                    if i == 0 and g < 3:
                        fold_tiles(g)
                    fold_tile(g, i)

                    # deferred work, spread so the PE never stalls in a clump
                    reduce_tile((m - 1) // 4, (m - 1) % 4)
                    if 5 <= m <= 12:
                        cumsum_round(m - 5)
                    if m == 12:
                        cs_b = cup.tile([32, 512], F32, tag="pc",
                                        name="cs_b")

            # ---- tail: flush cs_a (complete once m14's pieces are in),
            # then the last m-tile's colsums into cs_b ----
            csums_sb = smp.tile([32, 512], F32, tag="csums_sb")
            nc.vector.tensor_copy(csums_sb[:], cs_ps[:])
            nc.sync.dma_start(csums_d, csums_sb[:])
            reduce_tile(3, 3)
            csb_sb = smp.tile([32, 512], F32, tag="csb_sb")
            nc.vector.tensor_copy(csb_sb[:], cs_b[:, 0:512])
            nc.sync.dma_start(csumsb_d, csb_sb[:])
            nc.sync.dma_start(esums_d, esums_sb[:])

    nc.compile()
    return nc


def _consts():
    import ml_dtypes
    kk = np.arange(128)
    triu = (kk[:, None] <= kk[None, :]).astype(ml_dtypes.bfloat16)
    cc = np.arange(16)
    nn = np.arange(L) // 128
    ltw = (cc[:, None] < nn[None, :]).astype(ml_dtypes.bfloat16)
    ohwb = np.zeros((128, 63), np.float32)
    ohwb[:, 31] = 1.0
    return triu, ltw, ohwb.astype(ml_dtypes.bfloat16)


def prepare_in_maps(queries, keys, values):
    import ml_dtypes
    BF = ml_dtypes.bfloat16
    F8 = ml_dtypes.float8_e4m3fn
    q = np.ascontiguousarray(queries, dtype=np.float32).reshape(B, L, C)
    k = np.ascontiguousarray(keys, dtype=np.float32).reshape(B, L, C)
    v = np.ascontiguousarray(values, dtype=np.float32)            # [B,L,H,E]

    triu, ltw, ohwb = _consts()

    in_maps = []
    for c in range(NCORES):
        b, half = c // 2, c % 2
        Zb = np.concatenate([q[b], k[b]], axis=0)                 # [4096, 512]
        own = Zb[L * half:L * half + L]
        oth = Zb[L * (1 - half):L * (1 - half) + L]
        # rotate other-half 512-blocks by `half` so the checkerboard covers
        # complementary cross sub-blocks on the two cores of a batch
        oth = np.concatenate(
            [oth[512 * ((i + half) % 4):512 * ((i + half) % 4) + 512]
             for i in (0, 2, 1, 3)], axis=0)
        z8 = np.concatenate([own, oth], axis=0).astype(F8)        # [4096, 512]
        # zdr[j][p, kt, t] = z8[t, 256j + 128kt + p]
        zdr = np.zeros((2, 128, 2, T2), dtype=F8)
        for j in range(2):
            for kt in range(2):
                c0 = 256 * j + 128 * kt
                zdr[j, :, kt, :] = z8[:, c0:c0 + 128].T
        # diagonal cancel: dcan[c, 0, r] = delta_{c, r%128} |z8_r|^2 / 64
        d = (z8[:L].astype(np.float32) ** 2).sum(axis=1) / 64.0   # [2048]
        dcan = np.zeros((128, 2, L), dtype=F8)
        rr_ = np.arange(L)
        dcan[rr_ % 128, 0, rr_] = d.astype(F8)
        negi = np.zeros((128, 2, 128), dtype=F8)
        negi[np.arange(128), 0, np.arange(128)] = -64.0

        vp = np.ascontiguousarray(
            v[b].reshape(16, 128, H, E)[:, :, 4 * half:4 * half + 4, :]
            .transpose(1, 0, 2, 3))                               # [128,16,4,64]
        in_maps.append({
            "zdr": zdr, "dcan": dcan, "negi": negi,
            "vp": vp.astype(BF),
            "triu": triu, "ltw": ltw, "ohwb": ohwb,
        })
    return in_maps


def get_nc():
    if "nc" not in _CACHED:
        _CACHED["nc"] = _build_nc()
    return _CACHED["nc"]


def kernel(queries, keys, values, attn_mask):
    global LAST_RUN
    nc = get_nc()
    in_maps = prepare_in_maps(queries, keys, values)

    res = run_bass_kernel_spmd(nc, in_maps, list(range(NCORES)))
    LAST_RUN = res
    return _host_combine(res.results, queries, keys)


def _host_combine(results, queries, keys):
    # ---- host combine (tiny, float64) ----
    srows = np.zeros((B, 2, L))
    for c in range(NCORES):
        b, half = c // 2, c % 2
        r = results[c]
        es = r["esums"].astype(np.float64)                        # [128, 16, 2]
        srow = (es[:, :, 0] + es[:, :, 1]).T.reshape(L).copy()    # r = 128m+p
        cs = r["csums"].astype(np.float64)                        # [32, 512]
        cs[12:14] = r["csumsb"].astype(np.float64)[12:14]
        for p_idx, (rr, ccol) in enumerate(PAIRS_RC):
            srow[512 * ccol:512 * ccol + 512] += cs[p_idx]
        for g in range(4):
            for ip in range(3):
                x0 = 128 * (ip + 1)
                srow[512 * g + x0:512 * g + 512] += cs[14 + 3 * g + ip, x0:]
        srows[b, half] += srow
        # cross checkerboard colsums belong to the sibling core's rows
        for rp in range(4):
            for hb in range(2):
                cact = ((rp % 2 + 2 * hb) + half) % 4
                srows[b, 1 - half, 512 * cact:512 * cact + 512] += \
                    cs[6 + 2 * rp + hb]
    lse_t_sum = (np.log(srows) + SHIFT).sum(axis=(0, 1))          # [L]

    # instance part: per-timestep 8x8 grams (0.2% of the FLOPs) + exact dots
    q = queries.astype(np.float32).reshape(B, L, C)
    k = keys.astype(np.float32).reshape(B, L, C)
    Zi = np.concatenate([q, k], axis=0).transpose(1, 0, 2)        # [L, 2B, C]
    e = np.einsum('tic,tjc->tij', Zi, Zi).astype(np.float64)      # [L, 8, 8]
    dots = np.stack([e[:, bb, B + bb] for bb in range(B)])        # [B, L]
    ii = np.arange(2 * B)
    e[:, ii, ii] = -np.inf
    m = e.max(axis=2, keepdims=True)
    li_sum = (np.log(np.exp(e - m).sum(axis=2)) + m[..., 0]).sum(axis=1)

    corr_mean = (li_sum + lse_t_sum) / 16.0 - dots.mean(axis=0)
    index = np.argsort(-corr_mean, kind="stable")[:TOPK]

    out = np.empty((B, H, L, E), np.float32)
    for c in range(NCORES):
        b, half = c // 2, c % 2
        pl = results[c]["planes"].reshape(128, 16, 4, E)
        out[b, 4 * half:4 * half + 4] = (
            pl.transpose(2, 1, 0, 3).reshape(4, L, E))
    out[:, :, index, :] /= (index + 1).astype(np.float32)[None, None, :, None]
    return out


# revision 41
# speedup vs baseline: 1.0374x; 1.0011x over previous
"""Trainium2 SPMD kernel for nn_AutoCorrelation_loss_V (sparse_attention).

Math summary (reference reduces to this exactly):
  - scores are constant along the unmasked (causal) key range, so softmax is
    uniform over l <= index[k]: the output is cumsum(V, axis=L) with the 7
    selected rows divided by (idx+1).
  - the top-7 indices come from corr.mean(batch), where
      corr[b,t] = 0.25*(LSE_i1 + LSE_i2 + LSE_t1 + LSE_t2) - <q[b,t], k[b,t]>
    with LSE_t* = row-logsumexp (diag dropped) of the temporal Gram
    Z_b @ Z_b^T (Z_b = concat(q_b, k_b), [4096, 512]) and LSE_i* the row-LSE
    of the per-timestep 8x8 instance Gram.
  - only the top-7 RANKING feeds the output (corr values never do), so the
    temporal Gram (99.6% of the FLOPs) runs in fp8e4 with DoubleRow matmuls
    (0.5 cycles/row in the cost model, 4x cheaper than f32r) -- verified to
    preserve the exact top-7 with ~5x margin.

Sharding (8 cores): core c = (b = c//2, half = c%2), owning the 2048 Gram
rows [2048*half, +2048) of batch b:
  - own-half columns use upper-triangular symmetry, incl. 128-granular
    triangles inside the diagonal superblocks; the cross block is split
    checkerboard-style between the two cores of a batch. The true diagonal
    is subtracted exactly via a diag(|z8_r|^2/64) @ (-64*I) fp8 matmul.
  - ScalarE exps each Gram tile (bias=-SHIFT) into bf16 scr tiles -- the
    single elementwise pass over the half-Gram is the bottleneck engine.
  - row sums: DVE tensor_scalar accumulate over scr (4x bf16 mode);
    mirrored row sums: PE one-hot column-sum matmuls of scr (cross slices
    pre-folded 4->1 on DVE); combined on the host in float64.
  - cumsum of V planes (b, heads 4*half+0..3) via bf16 triangular-ones
    matmuls (chunk-local cumsum + chunk-sum carry), interleaved into PE
    slack; a PE warmup burst holds the p-state ramp off the critical path.
Host: assembles row LSEs, computes the tiny instance grams / exact dots
(0.4% of FLOPs), takes top-7, divides those rows by (idx+1) while writing
the [4, 8, 2048, 64] output.
"""

import sys

import numpy as np

sys.path.insert(0, "/opt/trn_rl_repo")

import concourse.bacc as bacc
import concourse.tile as tile
from concourse import mybir
from concourse.bass_utils import run_bass_kernel_spmd

F32 = mybir.dt.float32
BF16 = mybir.dt.bfloat16
FP8 = mybir.dt.float8e4
DR = mybir.MatmulPerfMode.DoubleRow
EXP = mybir.ActivationFunctionType.Exp
ADD = mybir.AluOpType.add

B, L, H, E = 4, 2048, 8, 64
C = H * E  # 512
T2 = 2 * L  # 4096
NCORES = 8
TOPK = 7  # int(1.0 * log(2048))
SHIFT = 100.0

PAIRS_RC = [(0, 1), (0, 2), (0, 3), (1, 2), (1, 3), (2, 3)]

LAST_RUN = None  # BassKernelResults of the most recent launch (for test.py)
_CACHED = {}


def _build_nc():
    nc = bacc.Bacc("TRN2", target_bir_lowering=False, debug=False,
                   num_devices=NCORES)

    zdr_d = nc.dram_tensor("zdr", [2, 128, 2, T2], FP8, kind="ExternalInput").ap()
    dcan_d = nc.dram_tensor("dcan", [128, L], FP8, kind="ExternalInput").ap()
    negi_d = nc.dram_tensor("negi", [128, 128], FP8, kind="ExternalInput").ap()
    vp_d = nc.dram_tensor("vp", [128, 16, 4, E], BF16, kind="ExternalInput").ap()
    triu_d = nc.dram_tensor("triu", [128, 128], BF16, kind="ExternalInput").ap()
    ltw_d = nc.dram_tensor("ltw", [16, L], BF16, kind="ExternalInput").ap()
    ohwb_d = nc.dram_tensor("ohwb", [128, 63], BF16, kind="ExternalInput").ap()

    esums_d = nc.dram_tensor("esums", [128, 16, 2], F32, kind="ExternalOutput").ap()
    csums_d = nc.dram_tensor("csums", [32, 512], F32, kind="ExternalOutput").ap()
    csumsb_d = nc.dram_tensor("csumsb", [32, 512], F32, kind="ExternalOutput").ap()
    planes_d = nc.dram_tensor("planes", [128, 16, 256], F32, kind="ExternalOutput").ap()

    with tile.TileContext(nc) as tc:
        with tc.tile_pool(name="zin", bufs=1) as zp, \
             tc.tile_pool(name="const", bufs=1) as cp, \
             tc.tile_pool(name="small", bufs=1) as smp, \
             tc.tile_pool(name="scra", bufs=5) as sca, \
             tc.tile_pool(name="scrb", bufs=5) as scb, \
             tc.tile_pool(name="fold", bufs=1) as fop, \
             tc.tile_pool(name="junk", bufs=2) as jkp, \
             tc.tile_pool(name="pout", bufs=2) as pop, \
             tc.tile_pool(name="gram", bufs=2, space="PSUM") as gp, \
             tc.tile_pool(name="csp", bufs=1, space="PSUM") as csp, \
             tc.tile_pool(name="cum", bufs=1, space="PSUM") as cup:

            # ---- inputs, all on the SP HWDGE queue so the global DMA
            # device processes them in exactly this order ----
            warm_sb = cp.tile([128, 512], BF16, tag="warm")
            nc.gpsimd.memset(warm_sb[:], 0.0)
            bias_sb = cp.tile([128, 1], F32, tag="bias")
            nc.gpsimd.memset(bias_sb[:], -SHIFT)
            # dummy exp so Bacc's activation-table load runs during the
            # DMA-wait window instead of delaying the first real activation
            dummy_sb = cp.tile([128, 1], BF16, tag="dummy")
            nc.scalar.activation(dummy_sb[:], bias_sb[:], EXP, bias=bias_sb[:])

            zdr_sb = []
            for j in range(2):
                t = zp.tile([128, 2, T2], FP8, tag=f"zdr{j}", name=f"zdr{j}")
                zdr_sb.append(t)
            for j in range(2):  # own columns first
                nc.sync.dma_start(zdr_sb[j][:, :, 0:2048],
                                  zdr_d[j][:, :, 0:2048])
            dcan_sb = zp.tile([128, 2, L], FP8, tag="dcan")
            nc.gpsimd.memset(dcan_sb[:, 1, :], 0.0)
            nc.sync.dma_start(dcan_sb[:, 0, :], dcan_d)
            negi_sb = zp.tile([128, 2, 128], FP8, tag="negi")
            nc.gpsimd.memset(negi_sb[:, 1, :], 0.0)
            nc.sync.dma_start(negi_sb[:, 0, :], negi_d)
            for j in range(2):  # cross chunks of even superblocks
                nc.sync.dma_start(zdr_sb[j][:, :, 2048:3072],
                                  zdr_d[j][:, :, 2048:3072])
            ohwb_sb = cp.tile([128, 63], BF16, tag="ohwb")
            nc.sync.dma_start(ohwb_sb[:], ohwb_d)
            for j in range(2):  # cross chunks of odd superblocks
                nc.sync.dma_start(zdr_sb[j][:, :, 3072:4096],
                                  zdr_d[j][:, :, 3072:4096])
            vp_sb = zp.tile([128, 16, 4, E], BF16, tag="vp")
            nc.sync.dma_start(vp_sb[:], vp_d)
            triu_sb = cp.tile([128, 128], BF16, tag="triu")
            nc.sync.dma_start(triu_sb[:], triu_d)
            ltw_sb = cp.tile([16, L], BF16, tag="ltw")
            nc.sync.dma_start(ltw_sb[:], ltw_d)

            esums_sb = smp.tile([128, 16, 2], F32, tag="esums")
            nc.gpsimd.memset(esums_sb[:], 0.0)
            cs_ps = csp.tile([32, 512], F32, tag="csps")
            cs_b = None  # late-created [32, 512] in the cum bank for g=3 cross
            # PE warmup: ramp the p-state while zdr streams in (output is
            # never read; the slot is recycled by the first gram tile)
            ps_warm = gp.tile([128, 1536], F32, tag="gram", name="ps_warm")
            for _ in range(6):
                nc.tensor.matmul(ps_warm[:, 0:512], warm_sb[:, 0:128],
                                 warm_sb[:], start=True, stop=True,
                                 skip_group_check=True)
            cs_total = 42
            cs_state = {"n": 0}

            def cs_mm(row, rhs_ap, col0=0):
                # accumulate column sums of rhs into cs_ps[row, col0:]
                w = rhs_ap.shape[-1]
                nc.tensor.matmul(cs_ps[:, col0:col0 + w],
                                 ohwb_sb[:, 31 - row:63 - row], rhs_ap,
                                 start=cs_state["n"] == 0,
                                 stop=cs_state["n"] == cs_total - 1,
                                 skip_group_check=True)
                cs_state["n"] += 1

            csb_state = {"n": 0}

            def cs_mm_b(row, rhs_ap):
                # g=3 cross colsums into the late cs_b accumulator
                nc.tensor.matmul(cs_b[:, 0:512],
                                 ohwb_sb[:, 31 - row:63 - row], rhs_ap,
                                 start=csb_state["n"] == 0,
                                 stop=csb_state["n"] == 7,
                                 skip_group_check=True)
                csb_state["n"] += 1

            def gram_mms(ps, col0, lcol, rcol, w, cancel=False):
                # accumulate Z8[:, lcol:+128]^T @ Z8[:, rcol:+w] into
                # ps[:, col0:col0+w]; optionally subtract the true diagonal
                # (diag(|z8_r|^2/64) @ (-64 I), exact to ~|20| which the
                # exp(-SHIFT) bias flushes to zero)
                nc.tensor.matmul(ps[:, col0:col0 + w],
                                 zdr_sb[0][:, :, lcol:lcol + 128],
                                 zdr_sb[0][:, :, rcol:rcol + w],
                                 start=True, stop=False, perf_mode=DR)
                if cancel:
                    nc.tensor.matmul(ps[:, col0:col0 + 128],
                                     dcan_sb[:, :, lcol:lcol + 128],
                                     negi_sb[:],
                                     start=False, stop=False, perf_mode=DR,
                                     skip_group_check=True)
                nc.tensor.matmul(ps[:, col0:col0 + w],
                                 zdr_sb[1][:, :, lcol:lcol + 128],
                                 zdr_sb[1][:, :, rcol:rcol + w],
                                 start=False, stop=True, perf_mode=DR)

            def chunk_sums():
                # V chunk sums for the cumsum carries
                ps_sums = cup.tile([16, 256], F32, tag="pc", name="ps_sums")
                for n in range(16):
                    nc.tensor.matmul(ps_sums[:], ohwb_sb[:, 31 - n:47 - n],
                                     vp_sb[:, n], start=(n == 0),
                                     stop=(n == 15))
                sums_sb = smp.tile([16, 256], BF16, tag="sums_sb")
                nc.vector.tensor_copy(sums_sb[:], ps_sums[:])
                return sums_sb

            sums_sb = None

            def cumsum_round(nr):
                # chunks (2nr, 2nr+1) -> planes
                pc = cup.tile([128, 2, 256], F32, tag="pc", name="pc")
                for hh in range(2):
                    n = 2 * nr + hh
                    nc.tensor.matmul(pc[:, hh, :],
                                     ltw_sb[:, 128 * n:128 * n + 128],
                                     sums_sb[:], start=True, stop=False)
                    nc.tensor.matmul(pc[:, hh, :], triu_sb[:],
                                     vp_sb[:, n], start=False, stop=True)
                po = pop.tile([128, 2, 256], F32, tag="pout")
                nc.vector.tensor_copy(po[:], pc[:])
                nc.sync.dma_start(planes_d[:, 2 * nr:2 * nr + 2], po[:])

            # ---- the Gram m-loop ----
            # Tile column layouts (all matmul regions 512-bank-aligned; the
            # diag part sits at its natural [128i, 512) offset in bank 0):
            #   g=0: A = [chk1 | chk2 | chk3]          B = [diag | crossx2]
            #   g=1: A = [diag | chk2 | chk3]          B = [crossx2]
            #   g=2: A = [diag | chk3]                 B = [crossx2]
            #   g=3: A = [diag | crossx2]              (no B)
            sb_scr_all = {g: [] for g in range(4)}
            fold_state = {}

            def fold_tiles(g):
                # per-superblock bf16 cross-fold accumulator on DVE
                fc = fop.tile([128, 1024], BF16, tag=f"fc{g}", name=f"fc{g}")
                fold_state[g] = fc

            def fold_tile(g, i):
                # accumulate m-tile (g, i)'s cross slice (g=3 goes direct)
                if g == 3:
                    return
                scr_a, scr_b = sb_scr_all[g][i]
                fc = fold_state[g]
                cscr, co = (scr_b, 512) if g == 0 else (scr_b, 0)
                csrc = cscr[:, co:co + 1024]
                if i == 0:
                    nc.vector.tensor_copy(fc[:], csrc)
                else:
                    nc.vector.tensor_add(fc[:], fc[:], csrc)

            def reduce_tile(g, i):
                # diag partials and own-pair colsums direct from scr (PE);
                # the folded cross colsums at i == 3
                scr_a, scr_b = sb_scr_all[g][i]
                for n in range(g + 1, 4):
                    o = 512 * (n - 1) if g == 0 else 512 * (n - g)
                    cs_mm(PAIRS_RC.index((g, n)), scr_a[:, o:o + 512])
                if i < 3:
                    dscr = scr_b if g == 0 else scr_a
                    cs_mm(14 + 3 * g + i, dscr[:, 128 * i:512], col0=128 * i)
                if g == 3:
                    for hb in range(2):
                        cs_mm_b(6 + 2 * g + hb,
                                scr_a[:, 512 + 512 * hb:1024 + 512 * hb])
                elif i == 3:
                    fc = fold_state[g]
                    for hb in range(2):
                        cs_mm(6 + 2 * g + hb, fc[:, 512 * hb:512 * hb + 512])

            def emit_act(scr, ps, s, e, m, slot):
                nc.scalar.activation(scr[:, s:e], ps[:, s:e], EXP,
                                     bias=bias_sb[:])
                jk = jkp.tile([128, 1536], BF16, tag="jk", name="jk")
                nc.vector.tensor_scalar(jk[:, s:e], scr[:, s:e], 0.0, None,
                                        op0=ADD, op1=ADD,
                                        accum_out=esums_sb[:, m, slot:slot + 1])

            b0_ps = {}

            def emit_b0_cross(i):
                # g=0 B tile, cross half (early data: own + cr-even only)
                ps_b = gp.tile([128, 1536], F32, tag="gram", name="ps_b")
                for hb in range(2):
                    gram_mms(ps_b, 512 + 512 * hb, 128 * i,
                             2048 + 512 * hb, 512)
                b0_ps[i] = ps_b

            def emit_b0_diag(i):
                # g=0 B tile, dcan-gated diag piece + the activation
                ps_b = b0_ps[i]
                gram_mms(ps_b, 128 * i, 128 * i, 128 * i, 512 - 128 * i,
                         cancel=True)
                scr_b = scb.tile([128, 1536], BF16, tag="scr_b")
                emit_act(scr_b, ps_b, 128 * i, 1536, i, 1)
                sb_scr_all[0][i] = (sb_scr_all[0][i][0], scr_b)
                if i == 0:
                    fold_tiles(0)
                fold_tile(0, i)

            # g=0: own-only A tiles run ahead; each B tile's cross mms are
            # emitted early but its dcan-gated diag piece one A-tile later,
            # so the in-order PE never parks on the dcan DMA.
            for i in range(4):
                r0 = 128 * i
                ps_a = gp.tile([128, 1536], F32, tag="gram", name="ps_a")
                for n in range(1, 4):
                    gram_mms(ps_a, 512 * (n - 1), r0, 512 * n, 512)
                scr_a = sca.tile([128, 1536], BF16, tag="scr_a")
                emit_act(scr_a, ps_a, 0, 1536, i, 0)
                sb_scr_all[0].append((scr_a, None))
                if i == 1:
                    emit_b0_cross(0)
                elif i == 2:
                    emit_b0_diag(0)
                    emit_b0_cross(1)
                elif i == 3:
                    emit_b0_diag(1)
                    reduce_tile(0, 0)
            emit_b0_cross(2)
            emit_b0_diag(2)
            reduce_tile(0, 1)
            emit_b0_cross(3)
            emit_b0_diag(3)
            reduce_tile(0, 2)
            sums_sb = chunk_sums()

            for g in range(1, 4):
                cpos = [2 * (g % 2), 2 * (g % 2) + 1]  # stored cross slots
                sb_scr = sb_scr_all[g]
                for i in range(4):
                    m = 4 * g + i
                    r0 = 128 * m
                    dia_c = 512 * g + 128 * i  # first diag-part column
                    dia_w = 512 - 128 * i
                    ps_a = gp.tile([128, 1536], F32, tag="gram", name="ps_a")
                    ps_b = None
                    if g < 3:
                        gram_mms(ps_a, 128 * i, r0, dia_c, dia_w, cancel=True)
                        for n in range(g + 1, 4):
                            gram_mms(ps_a, 512 * (n - g), r0, 512 * n, 512)
                        s_a, e_a = 128 * i, 512 * (4 - g)
                        ps_b = gp.tile([128, 1536], F32, tag="gram", name="ps_b")
                        for hb in range(2):
                            gram_mms(ps_b, 512 * hb, r0,
                                     2048 + 512 * cpos[hb], 512)
                        s_b, e_b = 0, 1024
                    else:
                        gram_mms(ps_a, 128 * i, r0, dia_c, dia_w, cancel=True)
                        for hb in range(2):
                            gram_mms(ps_a, 512 + 512 * hb, r0,
                                     2048 + 512 * cpos[hb], 512)
                        s_a, e_a = 128 * i, 1536

                    scr_a = sca.tile([128, 1536], BF16, tag="scr_a")
                    emit_act(scr_a, ps_a, s_a, e_a, m, 0)
                    scr_b = None
                    if ps_b is not None:
                        scr_b = scb.tile([128, 1536], BF16, tag="scr_b")
                        emit_act(scr_b, ps_b, s_b, e_b, m, 1)
                    sb_scr.append((scr_a, scr_b))
                    if i == 0 and g < 3:
                        fold_tiles(g)
                    fold_tile(g, i)

                    # deferred work, spread so the PE never stalls in a clump
                    reduce_tile((m - 1) // 4, (m - 1) % 4)
                    if 5 <= m <= 12:
                        cumsum_round(m - 5)
                    if m == 12:
                        cs_b = cup.tile([32, 512], F32, tag="pc",
                                        name="cs_b")

            # ---- tail: flush cs_a (complete once m14's pieces are in),
            # then the last m-tile's colsums into cs_b ----
            csums_sb = smp.tile([32, 512], F32, tag="csums_sb")
            nc.vector.tensor_copy(csums_sb[:], cs_ps[:])
            nc.sync.dma_start(csums_d, csums_sb[:])
            reduce_tile(3, 3)
            csb_sb = smp.tile([32, 512], F32, tag="csb_sb")
            nc.vector.tensor_copy(csb_sb[:], cs_b[:, 0:512])
            nc.sync.dma_start(csumsb_d, csb_sb[:])
            nc.sync.dma_start(esums_d, esums_sb[:])

    nc.compile()
    return nc


def _consts():
    import ml_dtypes
    kk = np.arange(128)
    triu = (kk[:, None] <= kk[None, :]).astype(ml_dtypes.bfloat16)
    cc = np.arange(16)
    nn = np.arange(L) // 128
    ltw = (cc[:, None] < nn[None, :]).astype(ml_dtypes.bfloat16)
    ohwb = np.zeros((128, 63), np.float32)
    ohwb[:, 31] = 1.0
    return triu, ltw, ohwb.astype(ml_dtypes.bfloat16)


def prepare_in_maps(queries, keys, values):
    import ml_dtypes
    BF = ml_dtypes.bfloat16
    F8 = ml_dtypes.float8_e4m3fn
    q = np.ascontiguousarray(queries, dtype=np.float32).reshape(B, L, C)
    k = np.ascontiguousarray(keys, dtype=np.float32).reshape(B, L, C)
    v = np.ascontiguousarray(values, dtype=np.float32)            # [B,L,H,E]

    triu, ltw, ohwb = _consts()

    in_maps = []
    for c in range(NCORES):
        b, half = c // 2, c % 2
        Zb = np.concatenate([q[b], k[b]], axis=0)                 # [4096, 512]
        own = Zb[L * half:L * half + L]
        oth = Zb[L * (1 - half):L * (1 - half) + L]
        # rotate other-half 512-blocks by `half` so the checkerboard covers
        # complementary cross sub-blocks on the two cores of a batch
        oth = np.concatenate(
            [oth[512 * ((i + half) % 4):512 * ((i + half) % 4) + 512]
             for i in (0, 2, 1, 3)], axis=0)
        z8 = np.concatenate([own, oth], axis=0).astype(F8)        # [4096, 512]
        # zdr[j][p, kt, t] = z8[t, 256j + 128kt + p]
        zdr = np.zeros((2, 128, 2, T2), dtype=F8)
        for j in range(2):
            for kt in range(2):
                c0 = 256 * j + 128 * kt
                zdr[j, :, kt, :] = z8[:, c0:c0 + 128].T
        # diagonal cancel: dcan[c, r] = delta_{c, r%128} |z8_r|^2 / 64
        d = (z8[:L].astype(np.float32) ** 2).sum(axis=1) / 64.0   # [2048]
        dcan = np.zeros((128, L), dtype=F8)
        rr_ = np.arange(L)
        dcan[rr_ % 128, rr_] = d.astype(F8)
        negi = np.zeros((128, 128), dtype=F8)
        negi[np.arange(128), np.arange(128)] = -64.0

        vp = np.ascontiguousarray(
            v[b].reshape(16, 128, H, E)[:, :, 4 * half:4 * half + 4, :]
            .transpose(1, 0, 2, 3))                               # [128,16,4,64]
        in_maps.append({
            "zdr": zdr, "dcan": dcan, "negi": negi,
            "vp": vp.astype(BF),
            "triu": triu, "ltw": ltw, "ohwb": ohwb,
        })
    return in_maps


def get_nc():
    if "nc" not in _CACHED:
        _CACHED["nc"] = _build_nc()
    return _CACHED["nc"]


def kernel(queries, keys, values, attn_mask):
    global LAST_RUN
    nc = get_nc()
    in_maps = prepare_in_maps(queries, keys, values)

    res = run_bass_kernel_spmd(nc, in_maps, list(range(NCORES)))
    LAST_RUN = res
    return _host_combine(res.results, queries, keys)


def _host_combine(results, queries, keys):
    # ---- host combine (tiny, float64) ----
    srows = np.zeros((B, 2, L))
    for c in range(NCORES):
        b, half = c // 2, c % 2
        r = results[c]
        es = r["esums"].astype(np.float64)                        # [128, 16, 2]
        srow = (es[:, :, 0] + es[:, :, 1]).T.reshape(L).copy()    # r = 128m+p
        cs = r["csums"].astype(np.float64)                        # [32, 512]
        cs[12:14] = r["csumsb"].astype(np.float64)[12:14]
        for p_idx, (rr, ccol) in enumerate(PAIRS_RC):
            srow[512 * ccol:512 * ccol + 512] += cs[p_idx]
        for g in range(4):
            for ip in range(3):
                x0 = 128 * (ip + 1)
                srow[512 * g + x0:512 * g + 512] += cs[14 + 3 * g + ip, x0:]
        srows[b, half] += srow
        # cross checkerboard colsums belong to the sibling core's rows
        for rp in range(4):
            for hb in range(2):
                cact = ((rp % 2 + 2 * hb) + half) % 4
                srows[b, 1 - half, 512 * cact:512 * cact + 512] += \
                    cs[6 + 2 * rp + hb]
    lse_t_sum = (np.log(srows) + SHIFT).sum(axis=(0, 1))          # [L]

    # instance part: per-timestep 8x8 grams (0.2% of the FLOPs) + exact dots
    q = queries.astype(np.float32).reshape(B, L, C)
    k = keys.astype(np.float32).reshape(B, L, C)
    Zi = np.concatenate([q, k], axis=0).transpose(1, 0, 2)        # [L, 2B, C]
    e = np.einsum('tic,tjc->tij', Zi, Zi).astype(np.float64)      # [L, 8, 8]
    dots = np.stack([e[:, bb, B + bb] for bb in range(B)])        # [B, L]
    ii = np.arange(2 * B)
    e[:, ii, ii] = -np.inf
    m = e.max(axis=2, keepdims=True)
    li_sum = (np.log(np.exp(e - m).sum(axis=2)) + m[..., 0]).sum(axis=1)

    corr_mean = (li_sum + lse_t_sum) / 16.0 - dots.mean(axis=0)
    index = np.argsort(-corr_mean, kind="stable")[:TOPK]

    out = np.empty((B, H, L, E), np.float32)
    for c in range(NCORES):
        b, half = c // 2, c % 2
        pl = results[c]["planes"].reshape(128, 16, 4, E)
        out[b, 4 * half:4 * half + 4] = (
            pl.transpose(2, 1, 0, 3).reshape(4, L, E))
    out[:, :, index, :] /= (index + 1).astype(np.float32)[None, None, :, None]
    return out
